# revision 56
# baseline (speedup 1.0000x reference)
"""Trainium2 Bass kernel for nn_AtBatCell: GRU recurrence over a shared state
table with gather/scatter-add per timestep.

Strategy: steps touching disjoint table rows are independent, so the T=8192
sequential scan collapses into waves (levels of the row-dependency DAG).
The device runs the first DW waves (87.5% of steps) as fully-packed batches
of 128-step GRU chunks; the small high-level tail (steps whose row chains
are 3+ deep) is finished on the host together with the delta assembly the
host already performs.

Device schedule (per core, SPMD-identical):
 - wave 1: all rows are first touches -> host-packed contiguous stream,
   keep-steps (whose rows are re-read in wave 2) sorted into chunks 0/1.
 - wave 2: consumer rows are built ON THE PE as fresh2-prefill plus
   host-built 0/1 permutation-submatrix matmuls over chunk-0/1 dh
   (replaces the SWDGE scatter-add: no Q7 descriptor gen, no DMA wait).
 - diagonal split: U = 0.8*I + V. The 0.8*h term is one exact bf16
   identity matmul; V@h runs fp8 DoubleRow value-only (V is tiny, so the
   fp8 h-quantization error is attenuated ~40x) -> all three gates use
   fp8 with two K=256 matmuls each, and wave 2 reuses the same V blobs.
 - inputs are packed into a few large DRAM blobs split across both HWDGE
   rings (sync + scalar) in first-need order; dh ships in block batches.
 - 3-stage software pipelining (gates / rh-transpose / m+dh) keeps the
   PE FIFO from head-of-line blocking on ACT/DVE stages; wave-2 chunks
   interleave among late wave-1 chunks.
 - dummy matmuls on a DVE-memset tile bridge the HAM clock-gate warmup
   window so real matmuls run at 2.4 GHz from the first chunk."""
import os
import sys
for _p in ('/opt/trn_rl_repo', '/root/.axon_site/_ro/trn_rl_repo'):
    if os.path.isdir(_p) and _p not in sys.path:
        sys.path.insert(0, _p)

import collections
import numpy as np

SIT = 64
S = 256
S2 = 512
CHUNK = 128          # steps per compute chunk
NCORES = 8
DW = 2               # device waves; later waves are finished on host
K_CAP = [5, 2]       # chunks per wave per core
WARMUP_MM = 13       # dummy PE matmuls to warm the HAM clock gate


def _schedule(b, p, n_rows_total):
    T = len(b)
    bl = b.astype(np.int64)
    pl = p.astype(np.int64)
    last = np.zeros(n_rows_total, np.int64)
    lev = np.empty(T, np.int64)
    for t in range(T):
        lv = max(last[bl[t]], last[pl[t]]) + 1
        lev[t] = lv
        last[bl[t]] = lv
        last[pl[t]] = lv

    rowtouch = collections.defaultdict(list)
    for t in range(T):
        rowtouch[bl[t]].append((t, 0))
        rowtouch[pl[t]].append((t, 1))
    nxt = np.full((T, 2), -1, np.int64)
    first = np.zeros((T, 2), bool)
    for r, lst in rowtouch.items():
        first[lst[0][0], lst[0][1]] = True
        for (t1, s1), (t2, _) in zip(lst, lst[1:]):
            nxt[t1, s1] = t2

    prov_dev = lev <= DW
    free = np.zeros(T, bool)
    for t in range(T):
        if not prov_dev[t]:
            continue
        free[t] = all(
            nxt[t, s] < 0 or not prov_dev[nxt[t, s]] for s in (0, 1))

    # union-find over provisional device steps
    parent = np.arange(T)

    def find(a):
        while parent[a] != a:
            parent[a] = parent[parent[a]]
            a = parent[a]
        return a

    for t in range(T):
        if not prov_dev[t]:
            continue
        for s in (0, 1):
            t2 = nxt[t, s]
            if t2 >= 0 and prov_dev[t2]:
                ra, rb = find(t), find(t2)
                if ra != rb:
                    parent[ra] = rb
    comp = collections.defaultdict(list)
    for t in range(T):
        if prov_dev[t]:
            comp[find(t)].append(t)
    comps = sorted(comp.values(), key=lambda v: (-len(v), v[0]))

    # balance components across cores on (per-level counts, total)
    targets = np.zeros(DW + 1)
    cvecs = []
    for cv in comps:
        v = np.zeros(DW + 1)
        for t in cv:
            v[lev[t] - 1] += 1
        v[DW] = len(cv)
        cvecs.append(v)
        targets += v
    targets = np.maximum(targets / NCORES, 1e-9)
    loads = np.zeros((NCORES, DW + 1))
    cassign = {}
    for cv, v in zip(comps, cvecs):
        cidx = int(np.argmin(((loads + v) / targets).max(axis=1)))
        loads[cidx] += v
        cassign[cv[0]] = cidx

    # per-core wave placement: nonfree at their level, free fill remaining
    # capacity (any wave >= their level), overflow goes to the host tail
    wave_steps = [[[] for _ in range(DW)] for _ in range(NCORES)]
    for cv in comps:
        c = cassign[cv[0]]
        for t in cv:
            if not free[t]:
                wave_steps[c][lev[t] - 1].append(t)
    for c in range(NCORES):
        for w in range(DW):
            assert len(wave_steps[c][w]) <= K_CAP[w] * CHUNK, \
                f"core {c} wave {w}: nonfree overflow"
    for cv in comps:
        c = cassign[cv[0]]
        for t in cv:
            if not free[t]:
                continue
            for w in range(int(lev[t]) - 1, DW):
                if len(wave_steps[c][w]) < K_CAP[w] * CHUNK:
                    wave_steps[c][w].append(t)
                    break
            # else: host tail

    dev_mask = np.zeros(T, bool)
    for c in range(NCORES):
        for w in range(DW):
            for t in wave_steps[c][w]:
                dev_mask[t] = True

    keep = np.zeros((T, 2), bool)
    for t in range(T):
        if dev_mask[t]:
            for s in (0, 1):
                keep[t, s] = nxt[t, s] >= 0 and dev_mask[nxt[t, s]]

    # keep-steps first within each wave (scatter prefix)
    for c in range(NCORES):
        for w in range(DW):
            wave_steps[c][w].sort(key=lambda t: (not keep[t].any(), t))

    host_steps = np.nonzero(~dev_mask)[0]

    # invariants
    for r, lst in rowtouch.items():
        seen_host = False
        for (t, s) in lst:
            if dev_mask[t]:
                assert not seen_host
            else:
                seen_host = True
    for c in range(NCORES):
        for t in wave_steps[c][0]:
            assert first[t].all(), "non-fresh slot in wave 1"

    return dict(lev=lev, nxt=nxt, first=first, keep=keep,
                wave_steps=wave_steps, host_steps=host_steps,
                dev_mask=dev_mask)


def _build_host_data(x, b, p, Wz, Wr, Wh, Uz, Ur, Uh, bz, br, bh, table0):
    import ml_dtypes
    bf16 = ml_dtypes.bfloat16
    fp8 = ml_dtypes.float8_e4m3
    N = table0.shape[0]
    b = b.astype(np.int64)
    p = p.astype(np.int64)
    sch = _schedule(b, p, N)
    keep = sch['keep']
    wave_steps = sch['wave_steps']

    wave_chunks = list(K_CAP)
    k1, k2 = wave_chunks
    n_chunks = sum(wave_chunks)
    T_pad = n_chunks * CHUNK

    # scatter prefix: chunks holding keep-steps in wave 1..DW-1
    kc_wave = [0] * DW
    for w in range(DW - 1):
        mx = max(sum(1 for t in wave_steps[c][w] if keep[t].any())
                 for c in range(NCORES))
        kc_wave[w] = -(-mx // CHUNK)

    chunk_wave = np.repeat(np.arange(DW), wave_chunks)

    # per-core data
    per_core = []
    for c in range(NCORES):
        ob = np.full(T_pad, -1, np.int64)   # original row ids (host assembly)
        op = np.full(T_pad, -1, np.int64)
        x_c = np.zeros((T_pad, SIT), np.float32)
        bias_c = np.zeros(T_pad, np.float32)
        st_c = np.full(T_pad, -1, np.int64)
        j0 = 0
        for w, wc in enumerate(wave_chunks):
            ts = wave_steps[c][w]
            sl = slice(j0, j0 + len(ts))
            tsa = np.asarray(ts, np.int64)
            if len(ts):
                st_c[sl] = tsa
                ob[sl] = b[tsa]
                op[sl] = p[tsa]
                x_c[sl] = x[tsa]
                bias_c[sl] = 1.0
            j0 += wc * CHUNK
        dup = (ob == op) & (ob >= 0)
        assert not dup.any(), "dup steps unsupported with SBUF-dst scatter"
        per_core.append(dict(ob=ob, op=op, x_c=x_c, bias_c=bias_c, st=st_c))

    g0 = k1 * CHUNK
    kc = kc_wave[0]
    for c in range(NCORES):
        pc = per_core[c]
        ob, op, st = pc['ob'], pc['op'], pc['st']
        # wave-1 -> wave-2 hand-off as permutation-submatrix matmuls:
        # consumer rows = fresh2 prefill + sum_{prod chunk c0,side s}
        #   P[c0,s,w2c,side2] @ dh[c0][:, s, :].  P is host-built 0/1.
        slot_of = {int(t): q for q, t in enumerate(st) if t >= 0}
        sides = np.stack([ob, op], axis=1)
        pmats = {}
        for q in range(min(kc * CHUNK, T_pad)):
            t = int(st[q])
            if t < 0:
                continue
            cq, j = q // CHUNK, q % CHUNK
            for side in (0, 1):
                if not keep[t, side]:
                    continue
                t2 = int(sch['nxt'][t, side])
                q2 = slot_of[t2]
                assert q2 >= g0, "consumer not in wave 2"
                row = sides[q, side]
                if pc['ob'][q2] == row:
                    side2 = 0
                else:
                    assert pc['op'][q2] == row
                    side2 = 1
                w2c = q2 // CHUNK - k1
                key = (cq, side, w2c, side2)
                if key not in pmats:
                    pmats[key] = np.zeros((128, 128), np.float32)
                pmats[key][j, q2 % CHUNK] = 1.0
        # fixed union of active combos across cores is established below

        # xT with bias row: [65, T_pad]
        xT_c = np.zeros((SIT + 1, T_pad), np.float32)
        xT_c[:SIT] = pc['x_c'].T
        xT_c[SIT] = pc['bias_c']

        # wave-1 fresh rows: natural [slot, 2, S] layout per chunk, plus
        # fp8 transposed [state, slot] layout feeding the z/r DR matmuls
        fresh_c = np.zeros((k1, 128, 2, S), np.float32)
        for q in range(k1):
            sl = slice(q * CHUNK, (q + 1) * CHUNK)
            vb = ob[sl] >= 0
            vp = op[sl] >= 0
            fresh_c[q, vb, 0, :] = table0[ob[sl][vb]]
            fresh_c[q, vp, 1, :] = table0[op[sl][vp]]
        fresh_b16 = fresh_c.astype(bf16)
        freshT8 = np.zeros((k1, 128, 4, CHUNK), fp8)
        for q in range(k1):
            hcat = fresh_b16[q].reshape(128, S2).astype(np.float32)
            for k in range(4):
                freshT8[q, :, k, :] = \
                    hcat[:, CHUNK * k:CHUNK * (k + 1)].T.astype(fp8)

        # wave-2 H tile pre-fill: table0 value of every referenced row
        # (scattered dh accumulates on top to form the post-wave-1 value)
        fresh2 = np.zeros((128, 2 * k2, S), np.float32)
        for q in range(g0, T_pad):
            if st[q] < 0:
                continue
            cc = q // CHUNK - k1
            j = q % CHUNK
            fresh2[j, 2 * cc, :] = table0[ob[q]]
            fresh2[j, 2 * cc + 1, :] = table0[op[q]]

        per_core[c] = dict(pmats=pmats, xT=xT_c, fresh=fresh_b16,
                           freshT8=freshT8, fresh2=fresh2.astype(bf16),
                           ob=ob, op=op)

    WzT = np.concatenate([Wz.T, bz[None, :]], axis=0)
    WrT = np.concatenate([Wr.T, -br[None, :]], axis=0)
    WhT = np.concatenate([Wh.T, bh[None, :]], axis=0)

    def ut(U):
        return np.ascontiguousarray(U.T.reshape(4, 128, S2).transpose(1, 0, 2))

    # diagonal split: U @ h = 0.8*h (exact bf16 identity matmul) +
    # V @ h with V = U - 0.8I tiny -> fp8 value-only h is accurate
    I2 = np.eye(S2, dtype=np.float32)
    Vz8 = ut(Uz - 0.8 * I2).astype(fp8).reshape(128, 4 * S2)
    Vr8 = ut(Ur - 0.8 * I2).astype(fp8).reshape(128, 4 * S2)
    Vh8 = ut(Uh - 0.8 * I2).astype(fp8).reshape(128, 4 * S2)
    ident = np.eye(128, dtype=np.float32).astype(bf16)
    id_pair = np.concatenate(
        [ident, (0.8 * np.eye(128, dtype=np.float32)).astype(bf16)], axis=1)

    # ---- packed input blobs (per core where they differ) ----
    # tI [128, 128+128] bf16: identity | idx (int16 bitcast, padded)
    # tA [65, T_pad + 3*S2] bf16 : xT | WzT | WrT | WhT
    # t8a [128, 512 + 2*2048] fp8: freshT8_c0 | Uz8 | Ur8
    # tBa [128, 512 + 2048 + 512] bf16: fresh_c0 | UhT | fresh_c1
    # t8b [128, (k1-1)*512] fp8: freshT8_c1..c4
    # tBb [128, (k1-2)*512] bf16: fresh_c2..c4
    # tBc [128, 2*2048] bf16: UzT | UrT  (wave-2 z/r run in bf16)
    pkeys = sorted({k for pc in per_core for k in pc['pmats'].keys()})
    for c in range(NCORES):
        pc = per_core[c]
        tA = np.concatenate(
            [pc['xT'], WzT, WrT, WhT], axis=1).astype(bf16)
        f8 = pc['freshT8'].reshape(k1, 128, 4 * CHUNK)
        t8a = np.concatenate([f8[0], Vr8, Vz8, f8[1]], axis=1)
        t8b = np.ascontiguousarray(
            f8[2:].transpose(1, 0, 2).reshape(128, (k1 - 2) * 4 * CHUNK))
        fr = pc['fresh'].reshape(k1, 128, 2 * S)
        tI2 = np.concatenate(
            [id_pair.astype(np.float32), fr[0], fr[1]], axis=1).astype(bf16)
        tF1 = np.ascontiguousarray(
            fr[2:].transpose(1, 0, 2).reshape(128, (k1 - 2) * 2 * S)
        ).astype(bf16)
        pblob = np.zeros((128, max(len(pkeys), 1) * 128), np.float32)
        for i, key in enumerate(pkeys):
            if key in pc['pmats']:
                pblob[:, i * 128:(i + 1) * 128] = pc['pmats'][key]
        tF2 = np.concatenate(
            [pblob.astype(bf16),
             pc['fresh2'].reshape(128, 2 * k2 * S)], axis=1).astype(bf16)
        pc.update(tI=tI2, tA=tA, t8a=t8a, t8b=t8b, t8c=Vh8,
                  tF1=tF1, tF2=tF2)

    hd = dict(
        n_chunks=n_chunks, kc_wave=kc_wave, wave_chunks=wave_chunks,
        chunk_wave=chunk_wave, T_pad=T_pad, pkeys=pkeys,
        per_core=per_core,
        host_steps=sch['host_steps'], lev=sch['lev'],
        x=x, b=b, p=p, Wz=Wz, Wr=Wr, Wh=Wh, Uz=Uz, Ur=Ur, Uh=Uh,
        bz=bz, br=br, bh=bh,
    )
    return hd


def _build_nc(hd):
    import concourse.bacc as bacc
    import concourse.mybir as mybir
    import concourse.tile as tile

    n_chunks = hd['n_chunks']
    T_pad = hd['T_pad']
    k1, k2 = hd['wave_chunks']
    f32 = mybir.dt.float32
    bf16 = mybir.dt.bfloat16
    fp8 = mybir.dt.float8e4
    DR = mybir.MatmulPerfMode.DoubleRow

    nc = bacc.Bacc("TRN2", target_bir_lowering=False, debug=True)

    A_COLS = T_pad + 3 * S2
    nP = max(len(hd['pkeys']), 1)
    tI_in = nc.dram_tensor("tI", (128, 256 + 4 * S), bf16,
                           kind="ExternalInput")
    tA_in = nc.dram_tensor("tA", (SIT + 1, A_COLS), bf16, kind="ExternalInput")
    t8a_in = nc.dram_tensor("t8a", (128, 1024 + 2 * 4 * S2), fp8,
                            kind="ExternalInput")
    t8c_in = nc.dram_tensor("t8c", (128, 4 * S2), fp8, kind="ExternalInput")
    t8b_in = nc.dram_tensor("t8b", (128, (k1 - 2) * 4 * CHUNK), fp8,
                            kind="ExternalInput")
    tF1_in = nc.dram_tensor("tF1", (128, (k1 - 2) * 2 * S), bf16,
                            kind="ExternalInput")
    tF2_in = nc.dram_tensor("tF2", (128, nP * 128 + 2 * k2 * S), bf16,
                            kind="ExternalInput")

    dh_out = nc.dram_tensor("dh", (128, 2 * n_chunks, S), bf16,
                            kind="ExternalOutput")

    Sig = mybir.ActivationFunctionType.Sigmoid
    Tanh = mybir.ActivationFunctionType.Tanh

    chunk_wave = hd['chunk_wave']
    # dh store blocks: (chunk_start, n_chunks_in_block)
    blocks = [(0, 2), (2, 2), (4, 1), (5, 2)]
    blk_of = {}
    for bi, (cs, nb) in enumerate(blocks):
        for q in range(nb):
            blk_of[cs + q] = (bi, cs, nb, q)

    with tile.TileContext(nc) as tc:
        with tc.tile_pool(name="const", bufs=1) as cpool, \
             tc.tile_pool(name="dhb", bufs=2) as dhpool, \
             tc.tile_pool(name="work", bufs=4) as wpool, \
             tc.tile_pool(name="psA", bufs=1, space="PSUM") as psA, \
             tc.tile_pool(name="psZ", bufs=2, space="PSUM") as psZ, \
             tc.tile_pool(name="psR", bufs=2, space="PSUM") as psR, \
             tc.tile_pool(name="psM", bufs=2, space="PSUM") as psM, \
             tc.tile_pool(name="psH", bufs=1, space="PSUM") as psH:

            # ---- HAM warmup: a DVE-memset source lets the PE start
            # within ~1us of kernel entry, before any DMA lands ----
            wsrc = cpool.tile([128, S2], bf16, tag="wsrc")
            nc.vector.memset(wsrc[:], 1.0)
            warm_ps = psH.tile([128, S2], f32, tag="hg", name="warm")
            for _ in range(WARMUP_MM):
                nc.tensor.matmul(warm_ps[:], wsrc[:, 0:128], wsrc[:],
                                 start=True, stop=True)

            # ---- big packed loads, ordered by first-need time and
            # split across the two HWDGE rings (sync + scalar) ----
            tI = cpool.tile([128, 256 + 4 * S], bf16, tag="tI")
            nc.sync.dma_start(tI[:], tI_in[:])
            tA = cpool.tile([SIT + 1, A_COLS], bf16, tag="tA")
            nc.scalar.dma_start(tA[:], tA_in[:])
            t8a = cpool.tile([128, 1024 + 2 * 4 * S2], fp8, tag="t8a")
            nc.sync.dma_start(t8a[:], t8a_in[:])
            tF1 = cpool.tile([128, (k1 - 2) * 2 * S], bf16, tag="tF1")
            nc.scalar.dma_start(tF1[:], tF1_in[:])
            t8b = cpool.tile([128, (k1 - 2) * 4 * CHUNK], fp8, tag="t8b")
            nc.sync.dma_start(t8b[:], t8b_in[:])
            t8c = cpool.tile([128, 4 * S2], fp8, tag="t8c")
            nc.scalar.dma_start(t8c[:], t8c_in[:])
            tF2 = cpool.tile([128, nP * 128 + 2 * k2 * S], bf16, tag="tF2")
            nc.sync.dma_start(tF2[:], tF2_in[:])

            identb = tI[:, 0:128]
            id08 = tI[:, 128:256]

            # ---- input views ----
            xT = tA[:, 0:T_pad]
            WzTv = tA[:, T_pad:T_pad + S2]
            WrTv = tA[:, T_pad + S2:T_pad + 2 * S2]
            WhTv = tA[:, T_pad + 2 * S2:T_pad + 3 * S2]
            Vr8 = t8a[:, 512:512 + 4 * S2].rearrange(
                "p (k n) -> p k n", k=4)
            Vz8 = t8a[:, 512 + 4 * S2:512 + 8 * S2].rearrange(
                "p (k n) -> p k n", k=4)
            Vh8 = t8c[:].rearrange("p (k n) -> p k n", k=4)
            fT8 = {0: t8a[:, 0:512].rearrange("p (k c) -> p k c", k=4),
                   1: t8a[:, 512 + 8 * S2:].rearrange(
                       "p (k c) -> p k c", k=4)}
            for c in range(2, k1):
                fT8[c] = t8b[:, (c - 2) * 512:(c - 1) * 512].rearrange(
                    "p (k c) -> p k c", k=4)
            freshv = {0: tI[:, 256:768], 1: tI[:, 768:1280]}
            for c in range(2, k1):
                freshv[c] = tF1[:, (c - 2) * 512:(c - 1) * 512]
            Pv = {key: tF2[:, i * 128:(i + 1) * 128]
                  for i, key in enumerate(hd['pkeys'])}
            fresh2f = tF2[:, nP * 128:nP * 128 + 2 * k2 * S]

            dh_tiles = {}
            st = {c: {} for c in range(n_chunks)}

            def stage_A(c):
                """hg build (wave-2), ht, x-proj + z/r U matmuls, sigmoids,
                rh = r*h."""
                w = int(chunk_wave[c])
                s_ = st[c]
                if w == 0:
                    hg2 = freshv[c]
                    h8t = fT8[c]
                else:
                    cw = c - k1
                    # consumer rows = fresh2 prefill + permuted wave-1 dh,
                    # built by PE matmuls (exact in bf16; f32 accumulate)
                    hgp = psH.tile([128, S2], f32, tag="hg",
                                   name=f"hgp_{c}")
                    contrib = [key for key in hd['pkeys'] if key[2] == cw]
                    # every half-bank must be written by >=1 P matmul, else
                    # the DVE evacuation would read stale PSUM data
                    assert {k[3] for k in contrib} == {0, 1}, \
                        "wave-2 side without producer coverage"
                    # zero the bank via a cleared accumulation group, then
                    # P-matmul contributions; fresh2 prefill rides the DVE
                    # evacuation as an add (saves an N=512 inject matmul)
                    for ki, key in enumerate(contrib):
                        cq, sd, _, side2 = key
                        assert cq < 2, "P producer outside dh block 0"
                        dh_src = dh_tiles[0][:, 2 * cq + sd, :]
                        nc.tensor.matmul(
                            hgp[:, S * side2:S * (side2 + 1)],
                            Pv[key], dh_src,
                            start=(ki == 0), stop=(ki == len(contrib) - 1))
                    hgt = wpool.tile([128, S2], bf16, tag="hgw")
                    nc.vector.tensor_add(
                        hgt[:], hgp[:], fresh2f[:, S2 * cw:S2 * (cw + 1)])
                    hg2 = hgt[:]
                    # PE transpose of the wave-2 rows; reuses the hg PSUM
                    # bank (already copied out) so psA stays rht-only
                    hgp_bf = hgp[:].bitcast(bf16)
                    for k in range(4):
                        nc.tensor.transpose(
                            hgp_bf[:, CHUNK * k:CHUNK * (k + 1)],
                            hg2[:, CHUNK * k:CHUNK * (k + 1)],
                            identb)
                    ht8 = wpool.tile([128, 4, CHUNK], fp8, tag="ht8")
                    nc.vector.tensor_copy(
                        ht8[:], hgp_bf[:, 0:S2].rearrange(
                            "p (k c) -> p k c", k=4))
                    h8t = ht8[:]

                xt_c = xT[:, CHUNK * c:CHUNK * (c + 1)]
                zpre = psZ.tile([128, S2], f32, tag="zpre")
                rpre = psR.tile([128, S2], f32, tag="rpre")
                nc.tensor.matmul(rpre[:], xt_c, WrTv, start=True, stop=False)
                nc.tensor.matmul(zpre[:], xt_c, WzTv, start=True, stop=False)
                # diagonal term 0.8*h, exact in bf16 (id08 @ h)
                nc.tensor.matmul(rpre[:], id08, hg2, start=False, stop=False)
                nc.tensor.matmul(zpre[:], id08, hg2, start=False, stop=False)
                # V (= U - 0.8I) in fp8 DoubleRow: K=256 per matmul, 2/gate;
                # r first: sigmoid(r) gates the rh -> transpose -> m chain
                for i in range(2):
                    sl = slice(2 * i, 2 * i + 2)
                    nc.tensor.matmul(rpre[:], h8t[:, sl, :], Vr8[:, sl, :],
                                     start=False, stop=(i == 1), perf_mode=DR)
                for i in range(2):
                    sl = slice(2 * i, 2 * i + 2)
                    nc.tensor.matmul(zpre[:], h8t[:, sl, :], Vz8[:, sl, :],
                                     start=False, stop=(i == 1), perf_mode=DR)

                zc = wpool.tile([128, S2], bf16, tag="zc")
                r = wpool.tile([128, S2], bf16, tag="r")
                nc.scalar.activation(r[:], rpre[:], Sig)
                nc.scalar.activation(zc[:], zpre[:], Sig, scale=-1.0)  # 1-z
                rh = wpool.tile([128, S2], bf16, tag="rh")
                nc.vector.tensor_mul(rh[:], r[:], hg2)
                s_.update(hg2=hg2, xt_c=xt_c, zc=zc, rh=rh)

            def stage_B1(c):
                """rht = transpose(rh) -> SBUF (feeds the m matmuls)."""
                s_ = st[c]
                tr_ps_f = psA.tile([128, 4, CHUNK], f32, tag="tr",
                                   name=f"trp_{c}")
                s_['tr_ps'] = tr_ps_f[:].bitcast(bf16)
                rht_ps = s_['tr_ps'][:, :, CHUNK:2 * CHUNK]
                rh = s_['rh']
                for k in range(4):
                    nc.tensor.transpose(
                        rht_ps[:, k, :], rh[:, CHUNK * k:CHUNK * (k + 1)],
                        identb)
                rht8 = wpool.tile([128, 4, CHUNK], fp8, tag="rht8")
                nc.vector.tensor_copy(rht8[:], rht_ps)
                s_['rht8'] = rht8

            def stage_B2(c, split=False, use_psh=False):
                """m matmuls, tanh, dh = (1-z)*(m-h), store."""
                s_ = st[c]
                bi, cs, nb, qb = blk_of[c]
                if bi not in dh_tiles:
                    dh_tiles[bi] = dhpool.tile([128, 2 * nb, S], bf16,
                                               tag="dh", name=f"dhb_{bi}")
                dhb = dh_tiles[bi]
                dh_v = dhb[:, 2 * qb:2 * qb + 2, :]
                rht8 = s_['rht8']
                # the last chunk borrows the hg bank: psM's single buffer
                # would gate its m-group on the previous chunk's tanh
                pool = psH if use_psh else psM
                mpre = pool.tile([128, S2], f32,
                                 tag="hg" if use_psh else "mpre",
                                 name=f"mpre_{c}")
                nc.tensor.matmul(mpre[:], s_['xt_c'], WhTv,
                                 start=True, stop=False)
                # diagonal term 0.8*(r*h) exact in bf16
                nc.tensor.matmul(mpre[:], id08, s_['rh'][:],
                                 start=False, stop=False)
                for i in range(2):
                    sl = slice(2 * i, 2 * i + 2)
                    nc.tensor.matmul(mpre[:], rht8[:, sl, :], Vh8[:, sl, :],
                                     start=False, stop=(i == 1),
                                     perf_mode=DR)
                m = wpool.tile([128, S2], bf16, tag="m")
                t1 = wpool.tile([128, S2], bf16, tag="t1")
                dh_flat = dh_v.rearrange("p a b -> p (a b)")
                if split:
                    # halve the tail chain so ACT/DVE/DMA pipeline and the
                    # final store issues earlier (used on the last chunks,
                    # which close their store blocks)
                    assert qb == nb - 1
                    for hi, hs in enumerate((slice(0, S), slice(S, S2))):
                        nc.scalar.activation(m[:, hs], mpre[:, hs], Tanh)
                        nc.vector.tensor_sub(t1[:, hs], m[:, hs],
                                             s_['hg2'][:, hs])
                        nc.vector.tensor_mul(dh_flat[:, hs], s_['zc'][:, hs],
                                             t1[:, hs])
                        lo = 0 if hi == 0 else 2 * qb + 1
                        hh = 2 * qb + 1 if hi == 0 else 2 * nb
                        # alternate HWDGE rings so the two half-stores
                        # don't serialize on one engine's issue cost; the
                        # last chunk flips the order so its final store
                        # lands on whichever ring is free
                        flip = (c == issue_order[-1])
                        eng = nc.scalar if (hi == 0) == flip else nc.sync
                        eng.dma_start(
                            dh_out[:, 2 * cs + lo:2 * cs + hh, :],
                            dhb[:, lo:hh, :])
                    return
                nc.scalar.activation(m[:], mpre[:], Tanh)
                nc.vector.tensor_sub(t1[:], m[:], s_['hg2'])
                nc.vector.tensor_mul(dh_flat, s_['zc'], t1[:])
                # ship deltas to host in per-block batches (sync HWDGE)
                if qb == nb - 1:
                    nc.sync.dma_start(
                        dh_out[:, 2 * cs:2 * (cs + nb), :], dhb[:])

            # wave-2 chunks (k1, k1+1) depend only on chunks 0/1; interleave
            # them among late wave-1 chunks. 3-stage software pipeline keeps
            # the PE FIFO from head-of-line blocking on ACT/DVE stages.
            issue_order = [0, 1, 2, 3, k1, k1 + 1, 4]
            assert sorted(issue_order) == list(range(n_chunks))
            for i, c in enumerate(issue_order):
                stage_A(c)
                if i >= 1:
                    stage_B1(issue_order[i - 1])
                if i >= 2:
                    stage_B2(issue_order[i - 2])

            # dependency-free filler matmuls slotted into the pipeline-drain
            # waits: the PE idles here on ACT/DVE chains long enough for the
            # HAM MID window to re-throttle the clock, which would make the
            # final real matmuls run at 1.2 GHz
            fill_ps = psH.tile([128, S2], f32, tag="hg", name="filler")

            def filler(n):
                for _ in range(n):
                    nc.tensor.matmul(fill_ps[:], wsrc[:, 0:128], wsrc[:],
                                     start=True, stop=True)

            stage_B1(issue_order[-1])
            filler(3)
            stage_B2(issue_order[-2], split=True)
            filler(3)
            stage_B2(issue_order[-1], split=True, use_psh=True)

    nc.compile()
    return nc


def _in_map(hd, core):
    pc = hd['per_core'][core]
    return {
        "tI": pc['tI'], "tA": pc['tA'], "t8a": pc['t8a'],
        "t8b": pc['t8b'], "t8c": pc['t8c'],
        "tF1": pc['tF1'], "tF2": pc['tF2'],
    }


def _run(hd, nc, trace=False):
    from concourse.bass_utils import run_bass_kernel_spmd
    return run_bass_kernel_spmd(nc, [_in_map(hd, c) for c in range(8)],
                                list(range(8)), trace=trace)


def _assemble(hd, dh_cores, table0):
    """Apply device deltas (rows never cross cores), then finish the tail
    waves on host (same-level steps never share a row -> batched GEMMs)."""
    n_chunks = hd['n_chunks']
    out = table0.astype(np.float32).copy()
    for cidx in range(8):
        dh = np.ascontiguousarray(dh_cores[cidx].transpose(1, 0, 2))
        dh = dh.reshape(n_chunks, 2, CHUNK, S).transpose(0, 2, 1, 3)
        dh = dh.reshape(hd['T_pad'] * 2, S)
        pc = hd['per_core'][cidx]
        rows = np.stack([pc['ob'], pc['op']], axis=1).reshape(-1)
        valid = rows >= 0
        np.add.at(out, rows[valid], dh[valid])

    hs = np.asarray(hd['host_steps'], np.int64)
    if len(hs):
        x, b, p = hd['x'], hd['b'], hd['p']
        Wz, Wr, Wh = hd['Wz'], hd['Wr'], hd['Wh']
        Uz, Ur, Uh = hd['Uz'], hd['Ur'], hd['Uh']
        bz, br, bh = hd['bz'], hd['br'], hd['bh']
        levs = hd['lev'][hs]
        for L in np.unique(levs):
            ts = hs[levs == L]
            H = np.concatenate([out[b[ts]], out[p[ts]]], axis=1)
            Z = 1 / (1 + np.exp(-(x[ts] @ Wz.T + H @ Uz.T + bz)))
            R = 1 / (1 + np.exp(-(x[ts] @ Wr.T + H @ Ur.T - br)))
            M = np.tanh(x[ts] @ Wh.T + (R * H) @ Uh.T + bh)
            dh = (1.0 - Z) * (M - H)
            np.add.at(out, b[ts], dh[:, :S])
            np.add.at(out, p[ts], dh[:, S:])
    return out


def kernel(**inputs):
    x = np.asarray(inputs['x'], dtype=np.float32)
    b = np.asarray(inputs['b'])
    p = np.asarray(inputs['p'])
    table0 = np.asarray(inputs['table0'], dtype=np.float32)

    hd = _build_host_data(
        x, b, p,
        np.asarray(inputs['Wz'], np.float32), np.asarray(inputs['Wr'], np.float32),
        np.asarray(inputs['Wh'], np.float32), np.asarray(inputs['Uz'], np.float32),
        np.asarray(inputs['Ur'], np.float32), np.asarray(inputs['Uh'], np.float32),
        np.asarray(inputs['bz'], np.float32), np.asarray(inputs['br'], np.float32),
        np.asarray(inputs['bh'], np.float32), table0)

    nc = _build_nc(hd)
    res = _run(hd, nc)
    dh_cores = [np.asarray(res.results[c]["dh"], np.float32) for c in range(8)]
    return _assemble(hd, dh_cores, table0)


if __name__ == "__main__":
    d = np.load('/tmp/ref_inputs.npz')
    inputs = {k: d[k] for k in d.files}
    got = kernel(**inputs)
    exp = np.load('/tmp/ref_out_np.npy')
    err = np.abs(got - exp).max()
    print("abs err:", err, "rel:", err / np.abs(exp).max())


# revision 57
# speedup vs baseline: 1.0328x; 1.0328x over previous
"""Trainium2 Bass kernel for nn_AtBatCell: GRU recurrence over a shared state
table with gather/scatter-add per timestep.

Strategy: steps touching disjoint table rows are independent, so the T=8192
sequential scan collapses into waves (levels of the row-dependency DAG).
The device runs the first DW waves (87.5% of steps) as fully-packed batches
of 128-step GRU chunks; the small high-level tail (steps whose row chains
are 3+ deep) is finished on the host together with the delta assembly the
host already performs.

Device schedule (per core, SPMD-identical):
 - wave 1: all rows are first touches -> host-packed contiguous stream,
   keep-steps (whose rows are re-read in wave 2) sorted into chunks 0/1.
 - wave 2: consumer rows are built ON THE PE as fresh2-prefill plus
   host-built 0/1 permutation-submatrix matmuls over chunk-0/1 dh
   (replaces the SWDGE scatter-add: no Q7 descriptor gen, no DMA wait).
 - diagonal split: U = 0.8*I + V. The 0.8*h term is one exact bf16
   identity matmul; V@h runs fp8 DoubleRow value-only (V is tiny, so the
   fp8 h-quantization error is attenuated ~40x) -> all three gates use
   fp8 with two K=256 matmuls each, and wave 2 reuses the same V blobs.
 - inputs are packed into a few large DRAM blobs split across both HWDGE
   rings (sync + scalar) in first-need order; dh ships in block batches.
 - 3-stage software pipelining (gates / rh-transpose / m+dh) keeps the
   PE FIFO from head-of-line blocking on ACT/DVE stages; wave-2 chunks
   interleave among late wave-1 chunks.
 - dummy matmuls on a DVE-memset tile bridge the HAM clock-gate warmup
   window so real matmuls run at 2.4 GHz from the first chunk."""
import os
import sys
for _p in ('/opt/trn_rl_repo', '/root/.axon_site/_ro/trn_rl_repo'):
    if os.path.isdir(_p) and _p not in sys.path:
        sys.path.insert(0, _p)

import collections
import numpy as np

SIT = 64
S = 256
S2 = 512
CHUNK = 128          # steps per compute chunk
NCORES = 8
DW = 2               # device waves; later waves are finished on host
K_CAP = [5, 2]       # chunks per wave per core
WARMUP_MM = 12       # dummy PE matmuls to warm the HAM clock gate


def _schedule(b, p, n_rows_total):
    T = len(b)
    bl = b.astype(np.int64)
    pl = p.astype(np.int64)
    last = np.zeros(n_rows_total, np.int64)
    lev = np.empty(T, np.int64)
    for t in range(T):
        lv = max(last[bl[t]], last[pl[t]]) + 1
        lev[t] = lv
        last[bl[t]] = lv
        last[pl[t]] = lv

    rowtouch = collections.defaultdict(list)
    for t in range(T):
        rowtouch[bl[t]].append((t, 0))
        rowtouch[pl[t]].append((t, 1))
    nxt = np.full((T, 2), -1, np.int64)
    first = np.zeros((T, 2), bool)
    for r, lst in rowtouch.items():
        first[lst[0][0], lst[0][1]] = True
        for (t1, s1), (t2, _) in zip(lst, lst[1:]):
            nxt[t1, s1] = t2

    prov_dev = lev <= DW
    free = np.zeros(T, bool)
    for t in range(T):
        if not prov_dev[t]:
            continue
        free[t] = all(
            nxt[t, s] < 0 or not prov_dev[nxt[t, s]] for s in (0, 1))

    # union-find over provisional device steps
    parent = np.arange(T)

    def find(a):
        while parent[a] != a:
            parent[a] = parent[parent[a]]
            a = parent[a]
        return a

    for t in range(T):
        if not prov_dev[t]:
            continue
        for s in (0, 1):
            t2 = nxt[t, s]
            if t2 >= 0 and prov_dev[t2]:
                ra, rb = find(t), find(t2)
                if ra != rb:
                    parent[ra] = rb
    comp = collections.defaultdict(list)
    for t in range(T):
        if prov_dev[t]:
            comp[find(t)].append(t)
    comps = sorted(comp.values(), key=lambda v: (-len(v), v[0]))

    # balance components across cores on (per-level counts, total)
    targets = np.zeros(DW + 1)
    cvecs = []
    for cv in comps:
        v = np.zeros(DW + 1)
        for t in cv:
            v[lev[t] - 1] += 1
        v[DW] = len(cv)
        cvecs.append(v)
        targets += v
    targets = np.maximum(targets / NCORES, 1e-9)
    loads = np.zeros((NCORES, DW + 1))
    cassign = {}
    for cv, v in zip(comps, cvecs):
        cidx = int(np.argmin(((loads + v) / targets).max(axis=1)))
        loads[cidx] += v
        cassign[cv[0]] = cidx

    # per-core wave placement: nonfree at their level, free fill remaining
    # capacity (any wave >= their level), overflow goes to the host tail
    wave_steps = [[[] for _ in range(DW)] for _ in range(NCORES)]
    for cv in comps:
        c = cassign[cv[0]]
        for t in cv:
            if not free[t]:
                wave_steps[c][lev[t] - 1].append(t)
    for c in range(NCORES):
        for w in range(DW):
            assert len(wave_steps[c][w]) <= K_CAP[w] * CHUNK, \
                f"core {c} wave {w}: nonfree overflow"
    for cv in comps:
        c = cassign[cv[0]]
        for t in cv:
            if not free[t]:
                continue
            for w in range(int(lev[t]) - 1, DW):
                if len(wave_steps[c][w]) < K_CAP[w] * CHUNK:
                    wave_steps[c][w].append(t)
                    break
            # else: host tail

    dev_mask = np.zeros(T, bool)
    for c in range(NCORES):
        for w in range(DW):
            for t in wave_steps[c][w]:
                dev_mask[t] = True

    keep = np.zeros((T, 2), bool)
    for t in range(T):
        if dev_mask[t]:
            for s in (0, 1):
                keep[t, s] = nxt[t, s] >= 0 and dev_mask[nxt[t, s]]

    # keep-steps first within each wave (scatter prefix)
    for c in range(NCORES):
        for w in range(DW):
            wave_steps[c][w].sort(key=lambda t: (not keep[t].any(), t))

    host_steps = np.nonzero(~dev_mask)[0]

    # invariants
    for r, lst in rowtouch.items():
        seen_host = False
        for (t, s) in lst:
            if dev_mask[t]:
                assert not seen_host
            else:
                seen_host = True
    for c in range(NCORES):
        for t in wave_steps[c][0]:
            assert first[t].all(), "non-fresh slot in wave 1"

    return dict(lev=lev, nxt=nxt, first=first, keep=keep,
                wave_steps=wave_steps, host_steps=host_steps,
                dev_mask=dev_mask)


def _build_host_data(x, b, p, Wz, Wr, Wh, Uz, Ur, Uh, bz, br, bh, table0):
    import ml_dtypes
    bf16 = ml_dtypes.bfloat16
    fp8 = ml_dtypes.float8_e4m3
    N = table0.shape[0]
    b = b.astype(np.int64)
    p = p.astype(np.int64)
    sch = _schedule(b, p, N)
    keep = sch['keep']
    wave_steps = sch['wave_steps']

    wave_chunks = list(K_CAP)
    k1, k2 = wave_chunks
    n_chunks = sum(wave_chunks)
    T_pad = n_chunks * CHUNK

    # scatter prefix: chunks holding keep-steps in wave 1..DW-1
    kc_wave = [0] * DW
    for w in range(DW - 1):
        mx = max(sum(1 for t in wave_steps[c][w] if keep[t].any())
                 for c in range(NCORES))
        kc_wave[w] = -(-mx // CHUNK)

    chunk_wave = np.repeat(np.arange(DW), wave_chunks)

    # per-core data
    per_core = []
    for c in range(NCORES):
        ob = np.full(T_pad, -1, np.int64)   # original row ids (host assembly)
        op = np.full(T_pad, -1, np.int64)
        x_c = np.zeros((T_pad, SIT), np.float32)
        bias_c = np.zeros(T_pad, np.float32)
        st_c = np.full(T_pad, -1, np.int64)
        j0 = 0
        for w, wc in enumerate(wave_chunks):
            ts = wave_steps[c][w]
            sl = slice(j0, j0 + len(ts))
            tsa = np.asarray(ts, np.int64)
            if len(ts):
                st_c[sl] = tsa
                ob[sl] = b[tsa]
                op[sl] = p[tsa]
                x_c[sl] = x[tsa]
                bias_c[sl] = 1.0
            j0 += wc * CHUNK
        dup = (ob == op) & (ob >= 0)
        assert not dup.any(), "dup steps unsupported with SBUF-dst scatter"
        per_core.append(dict(ob=ob, op=op, x_c=x_c, bias_c=bias_c, st=st_c))

    g0 = k1 * CHUNK
    kc = kc_wave[0]
    for c in range(NCORES):
        pc = per_core[c]
        ob, op, st = pc['ob'], pc['op'], pc['st']
        # wave-1 -> wave-2 hand-off as permutation-submatrix matmuls:
        # consumer rows = fresh2 prefill + sum_{prod chunk c0,side s}
        #   P[c0,s,w2c,side2] @ dh[c0][:, s, :].  P is host-built 0/1.
        slot_of = {int(t): q for q, t in enumerate(st) if t >= 0}
        sides = np.stack([ob, op], axis=1)
        pmats = {}
        for q in range(min(kc * CHUNK, T_pad)):
            t = int(st[q])
            if t < 0:
                continue
            cq, j = q // CHUNK, q % CHUNK
            for side in (0, 1):
                if not keep[t, side]:
                    continue
                t2 = int(sch['nxt'][t, side])
                q2 = slot_of[t2]
                assert q2 >= g0, "consumer not in wave 2"
                row = sides[q, side]
                if pc['ob'][q2] == row:
                    side2 = 0
                else:
                    assert pc['op'][q2] == row
                    side2 = 1
                w2c = q2 // CHUNK - k1
                key = (cq, side, w2c, side2)
                if key not in pmats:
                    pmats[key] = np.zeros((128, 128), np.float32)
                pmats[key][j, q2 % CHUNK] = 1.0
        # fixed union of active combos across cores is established below

        # xT with bias row: [65, T_pad]
        xT_c = np.zeros((SIT + 1, T_pad), np.float32)
        xT_c[:SIT] = pc['x_c'].T
        xT_c[SIT] = pc['bias_c']

        # wave-1 fresh rows: natural [slot, 2, S] layout per chunk, plus
        # fp8 transposed [state, slot] layout feeding the z/r DR matmuls
        fresh_c = np.zeros((k1, 128, 2, S), np.float32)
        for q in range(k1):
            sl = slice(q * CHUNK, (q + 1) * CHUNK)
            vb = ob[sl] >= 0
            vp = op[sl] >= 0
            fresh_c[q, vb, 0, :] = table0[ob[sl][vb]]
            fresh_c[q, vp, 1, :] = table0[op[sl][vp]]
        fresh_b16 = fresh_c.astype(bf16)
        freshT8 = np.zeros((k1, 128, 4, CHUNK), fp8)
        for q in range(k1):
            hcat = fresh_b16[q].reshape(128, S2).astype(np.float32)
            for k in range(4):
                freshT8[q, :, k, :] = \
                    hcat[:, CHUNK * k:CHUNK * (k + 1)].T.astype(fp8)

        # wave-2 H tile pre-fill: table0 value of every referenced row
        # (scattered dh accumulates on top to form the post-wave-1 value)
        fresh2 = np.zeros((128, 2 * k2, S), np.float32)
        for q in range(g0, T_pad):
            if st[q] < 0:
                continue
            cc = q // CHUNK - k1
            j = q % CHUNK
            fresh2[j, 2 * cc, :] = table0[ob[q]]
            fresh2[j, 2 * cc + 1, :] = table0[op[q]]

        per_core[c] = dict(pmats=pmats, xT=xT_c, fresh=fresh_b16,
                           freshT8=freshT8, fresh2=fresh2.astype(bf16),
                           ob=ob, op=op)

    WzT = np.concatenate([Wz.T, bz[None, :]], axis=0)
    WrT = np.concatenate([Wr.T, -br[None, :]], axis=0)
    WhT = np.concatenate([Wh.T, bh[None, :]], axis=0)

    def ut(U):
        return np.ascontiguousarray(U.T.reshape(4, 128, S2).transpose(1, 0, 2))

    # diagonal split: U @ h = 0.8*h (exact bf16 identity matmul) +
    # V @ h with V = U - 0.8I tiny -> fp8 value-only h is accurate
    I2 = np.eye(S2, dtype=np.float32)
    Vz8 = ut(Uz - 0.8 * I2).astype(fp8).reshape(128, 4 * S2)
    Vr8 = ut(Ur - 0.8 * I2).astype(fp8).reshape(128, 4 * S2)
    Vh8 = ut(Uh - 0.8 * I2).astype(fp8).reshape(128, 4 * S2)
    ident = np.eye(128, dtype=np.float32).astype(bf16)
    id_pair = np.concatenate(
        [ident, (0.8 * np.eye(128, dtype=np.float32)).astype(bf16)], axis=1)

    # ---- packed input blobs (per core where they differ) ----
    # tI [128, 128+128] bf16: identity | idx (int16 bitcast, padded)
    # tA [65, T_pad + 3*S2] bf16 : xT | WzT | WrT | WhT
    # t8a [128, 512 + 2*2048] fp8: freshT8_c0 | Uz8 | Ur8
    # tBa [128, 512 + 2048 + 512] bf16: fresh_c0 | UhT | fresh_c1
    # t8b [128, (k1-1)*512] fp8: freshT8_c1..c4
    # tBb [128, (k1-2)*512] bf16: fresh_c2..c4
    # tBc [128, 2*2048] bf16: UzT | UrT  (wave-2 z/r run in bf16)
    pkeys = sorted({k for pc in per_core for k in pc['pmats'].keys()})
    for c in range(NCORES):
        pc = per_core[c]
        tA = np.concatenate(
            [pc['xT'], WzT, WrT, WhT], axis=1).astype(bf16)
        f8 = pc['freshT8'].reshape(k1, 128, 4 * CHUNK)
        t8a = np.concatenate([f8[0], Vr8, Vz8, f8[1]], axis=1)
        t8b = np.ascontiguousarray(
            f8[2:].transpose(1, 0, 2).reshape(128, (k1 - 2) * 4 * CHUNK))
        fr = pc['fresh'].reshape(k1, 128, 2 * S)
        tI2 = np.concatenate(
            [id_pair.astype(np.float32), fr[0], fr[1]], axis=1).astype(bf16)
        tF1 = np.ascontiguousarray(
            fr[2:].transpose(1, 0, 2).reshape(128, (k1 - 2) * 2 * S)
        ).astype(bf16)
        pblob = np.zeros((128, max(len(pkeys), 1) * 128), np.float32)
        for i, key in enumerate(pkeys):
            if key in pc['pmats']:
                pblob[:, i * 128:(i + 1) * 128] = pc['pmats'][key]
        tF2 = np.concatenate(
            [pblob.astype(bf16),
             pc['fresh2'].reshape(128, 2 * k2 * S)], axis=1).astype(bf16)
        pc.update(tI=tI2, tA=tA, t8a=t8a, t8b=t8b, t8c=Vh8,
                  tF1=tF1, tF2=tF2)

    hd = dict(
        n_chunks=n_chunks, kc_wave=kc_wave, wave_chunks=wave_chunks,
        chunk_wave=chunk_wave, T_pad=T_pad, pkeys=pkeys,
        per_core=per_core,
        host_steps=sch['host_steps'], lev=sch['lev'],
        x=x, b=b, p=p, Wz=Wz, Wr=Wr, Wh=Wh, Uz=Uz, Ur=Ur, Uh=Uh,
        bz=bz, br=br, bh=bh,
    )
    return hd


def _build_nc(hd):
    import concourse.bacc as bacc
    import concourse.mybir as mybir
    import concourse.tile as tile

    n_chunks = hd['n_chunks']
    T_pad = hd['T_pad']
    k1, k2 = hd['wave_chunks']
    f32 = mybir.dt.float32
    bf16 = mybir.dt.bfloat16
    fp8 = mybir.dt.float8e4
    DR = mybir.MatmulPerfMode.DoubleRow

    nc = bacc.Bacc("TRN2", target_bir_lowering=False, debug=True)

    A_COLS = T_pad + 3 * S2
    nP = max(len(hd['pkeys']), 1)
    tI_in = nc.dram_tensor("tI", (128, 256 + 4 * S), bf16,
                           kind="ExternalInput")
    tA_in = nc.dram_tensor("tA", (SIT + 1, A_COLS), bf16, kind="ExternalInput")
    t8a_in = nc.dram_tensor("t8a", (128, 1024 + 2 * 4 * S2), fp8,
                            kind="ExternalInput")
    t8c_in = nc.dram_tensor("t8c", (128, 4 * S2), fp8, kind="ExternalInput")
    t8b_in = nc.dram_tensor("t8b", (128, (k1 - 2) * 4 * CHUNK), fp8,
                            kind="ExternalInput")
    tF1_in = nc.dram_tensor("tF1", (128, (k1 - 2) * 2 * S), bf16,
                            kind="ExternalInput")
    tF2_in = nc.dram_tensor("tF2", (128, nP * 128 + 2 * k2 * S), bf16,
                            kind="ExternalInput")

    dh_out = nc.dram_tensor("dh", (128, 2 * n_chunks, S), bf16,
                            kind="ExternalOutput")

    Sig = mybir.ActivationFunctionType.Sigmoid
    Tanh = mybir.ActivationFunctionType.Tanh

    chunk_wave = hd['chunk_wave']
    # dh store blocks: (chunk_start, n_chunks_in_block)
    blocks = [(0, 2), (2, 2), (4, 1), (5, 2)]
    blk_of = {}
    for bi, (cs, nb) in enumerate(blocks):
        for q in range(nb):
            blk_of[cs + q] = (bi, cs, nb, q)

    with tile.TileContext(nc) as tc:
        with tc.tile_pool(name="const", bufs=1) as cpool, \
             tc.tile_pool(name="dhb", bufs=2) as dhpool, \
             tc.tile_pool(name="work", bufs=4) as wpool, \
             tc.tile_pool(name="psA", bufs=1, space="PSUM") as psA, \
             tc.tile_pool(name="psZ", bufs=2, space="PSUM") as psZ, \
             tc.tile_pool(name="psR", bufs=2, space="PSUM") as psR, \
             tc.tile_pool(name="psM", bufs=2, space="PSUM") as psM, \
             tc.tile_pool(name="psH", bufs=1, space="PSUM") as psH:

            # ---- HAM warmup: a DVE-memset source lets the PE start
            # within ~1us of kernel entry, before any DMA lands ----
            wsrc = cpool.tile([128, S2], bf16, tag="wsrc")
            nc.vector.memset(wsrc[:], 1.0)
            warm_ps = psH.tile([128, S2], f32, tag="hg", name="warm")
            for _ in range(WARMUP_MM):
                nc.tensor.matmul(warm_ps[:], wsrc[:, 0:128], wsrc[:],
                                 start=True, stop=True)

            # ---- big packed loads, ordered by first-need time and
            # split across the two HWDGE rings (sync + scalar) ----
            tI = cpool.tile([128, 256 + 4 * S], bf16, tag="tI")
            nc.sync.dma_start(tI[:], tI_in[:])
            tA = cpool.tile([SIT + 1, A_COLS], bf16, tag="tA")
            nc.scalar.dma_start(tA[:], tA_in[:])
            t8a = cpool.tile([128, 1024 + 2 * 4 * S2], fp8, tag="t8a")
            nc.sync.dma_start(t8a[:], t8a_in[:])
            tF1 = cpool.tile([128, (k1 - 2) * 2 * S], bf16, tag="tF1")
            nc.scalar.dma_start(tF1[:], tF1_in[:])
            t8b = cpool.tile([128, (k1 - 2) * 4 * CHUNK], fp8, tag="t8b")
            nc.sync.dma_start(t8b[:], t8b_in[:])
            t8c = cpool.tile([128, 4 * S2], fp8, tag="t8c")
            nc.scalar.dma_start(t8c[:], t8c_in[:])
            tF2 = cpool.tile([128, nP * 128 + 2 * k2 * S], bf16, tag="tF2")
            nc.sync.dma_start(tF2[:], tF2_in[:])

            identb = tI[:, 0:128]
            id08 = tI[:, 128:256]

            # ---- input views ----
            xT = tA[:, 0:T_pad]
            WzTv = tA[:, T_pad:T_pad + S2]
            WrTv = tA[:, T_pad + S2:T_pad + 2 * S2]
            WhTv = tA[:, T_pad + 2 * S2:T_pad + 3 * S2]
            Vr8 = t8a[:, 512:512 + 4 * S2].rearrange(
                "p (k n) -> p k n", k=4)
            Vz8 = t8a[:, 512 + 4 * S2:512 + 8 * S2].rearrange(
                "p (k n) -> p k n", k=4)
            Vh8 = t8c[:].rearrange("p (k n) -> p k n", k=4)
            fT8 = {0: t8a[:, 0:512].rearrange("p (k c) -> p k c", k=4),
                   1: t8a[:, 512 + 8 * S2:].rearrange(
                       "p (k c) -> p k c", k=4)}
            for c in range(2, k1):
                fT8[c] = t8b[:, (c - 2) * 512:(c - 1) * 512].rearrange(
                    "p (k c) -> p k c", k=4)
            freshv = {0: tI[:, 256:768], 1: tI[:, 768:1280]}
            for c in range(2, k1):
                freshv[c] = tF1[:, (c - 2) * 512:(c - 1) * 512]
            Pv = {key: tF2[:, i * 128:(i + 1) * 128]
                  for i, key in enumerate(hd['pkeys'])}
            fresh2f = tF2[:, nP * 128:nP * 128 + 2 * k2 * S]

            dh_tiles = {}
            st = {c: {} for c in range(n_chunks)}

            def stage_A(c):
                """hg build (wave-2), ht, x-proj + z/r U matmuls, sigmoids,
                rh = r*h."""
                w = int(chunk_wave[c])
                s_ = st[c]
                if w == 0:
                    hg2 = freshv[c]
                    h8t = fT8[c]
                else:
                    cw = c - k1
                    # consumer rows = fresh2 prefill + permuted wave-1 dh,
                    # built by PE matmuls (exact in bf16; f32 accumulate)
                    hgp = psH.tile([128, S2], f32, tag="hg",
                                   name=f"hgp_{c}")
                    contrib = [key for key in hd['pkeys'] if key[2] == cw]
                    # every half-bank must be written by >=1 P matmul, else
                    # the DVE evacuation would read stale PSUM data
                    assert {k[3] for k in contrib} == {0, 1}, \
                        "wave-2 side without producer coverage"
                    # zero the bank via a cleared accumulation group, then
                    # P-matmul contributions; fresh2 prefill rides the DVE
                    # evacuation as an add (saves an N=512 inject matmul)
                    for ki, key in enumerate(contrib):
                        cq, sd, _, side2 = key
                        assert cq < 2, "P producer outside dh block 0"
                        dh_src = dh_tiles[0][:, 2 * cq + sd, :]
                        nc.tensor.matmul(
                            hgp[:, S * side2:S * (side2 + 1)],
                            Pv[key], dh_src,
                            start=(ki == 0), stop=(ki == len(contrib) - 1))
                    hgt = wpool.tile([128, S2], bf16, tag="hgw")
                    nc.vector.tensor_add(
                        hgt[:], hgp[:], fresh2f[:, S2 * cw:S2 * (cw + 1)])
                    hg2 = hgt[:]
                    # PE transpose of the wave-2 rows; reuses the hg PSUM
                    # bank (already copied out) so psA stays rht-only
                    hgp_bf = hgp[:].bitcast(bf16)
                    for k in range(4):
                        nc.tensor.transpose(
                            hgp_bf[:, CHUNK * k:CHUNK * (k + 1)],
                            hg2[:, CHUNK * k:CHUNK * (k + 1)],
                            identb)
                    ht8 = wpool.tile([128, 4, CHUNK], fp8, tag="ht8")
                    nc.vector.tensor_copy(
                        ht8[:], hgp_bf[:, 0:S2].rearrange(
                            "p (k c) -> p k c", k=4))
                    h8t = ht8[:]

                xt_c = xT[:, CHUNK * c:CHUNK * (c + 1)]
                zpre = psZ.tile([128, S2], f32, tag="zpre")
                rpre = psR.tile([128, S2], f32, tag="rpre")
                nc.tensor.matmul(rpre[:], xt_c, WrTv, start=True, stop=False)
                nc.tensor.matmul(zpre[:], xt_c, WzTv, start=True, stop=False)
                # diagonal term 0.8*h, exact in bf16 (id08 @ h)
                nc.tensor.matmul(rpre[:], id08, hg2, start=False, stop=False)
                nc.tensor.matmul(zpre[:], id08, hg2, start=False, stop=False)
                # V (= U - 0.8I) in fp8 DoubleRow: K=256 per matmul, 2/gate;
                # r first: sigmoid(r) gates the rh -> transpose -> m chain
                for i in range(2):
                    sl = slice(2 * i, 2 * i + 2)
                    nc.tensor.matmul(rpre[:], h8t[:, sl, :], Vr8[:, sl, :],
                                     start=False, stop=(i == 1), perf_mode=DR)
                for i in range(2):
                    sl = slice(2 * i, 2 * i + 2)
                    nc.tensor.matmul(zpre[:], h8t[:, sl, :], Vz8[:, sl, :],
                                     start=False, stop=(i == 1), perf_mode=DR)

                zc = wpool.tile([128, S2], bf16, tag="zc")
                r = wpool.tile([128, S2], bf16, tag="r")
                nc.scalar.activation(r[:], rpre[:], Sig)
                nc.scalar.activation(zc[:], zpre[:], Sig, scale=-1.0)  # 1-z
                rh = wpool.tile([128, S2], bf16, tag="rh")
                nc.vector.tensor_mul(rh[:], r[:], hg2)
                s_.update(hg2=hg2, xt_c=xt_c, zc=zc, rh=rh)

            def stage_B1(c):
                """rht = transpose(rh) -> SBUF (feeds the m matmuls)."""
                s_ = st[c]
                tr_ps_f = psA.tile([128, 4, CHUNK], f32, tag="tr",
                                   name=f"trp_{c}")
                s_['tr_ps'] = tr_ps_f[:].bitcast(bf16)
                rht_ps = s_['tr_ps'][:, :, CHUNK:2 * CHUNK]
                rh = s_['rh']
                for k in range(4):
                    nc.tensor.transpose(
                        rht_ps[:, k, :], rh[:, CHUNK * k:CHUNK * (k + 1)],
                        identb)
                rht8 = wpool.tile([128, 4, CHUNK], fp8, tag="rht8")
                nc.vector.tensor_copy(rht8[:], rht_ps)
                s_['rht8'] = rht8

            def stage_B2(c, split=False, use_psh=False):
                """m matmuls, tanh, dh = (1-z)*(m-h), store."""
                s_ = st[c]
                bi, cs, nb, qb = blk_of[c]
                if bi not in dh_tiles:
                    dh_tiles[bi] = dhpool.tile([128, 2 * nb, S], bf16,
                                               tag="dh", name=f"dhb_{bi}")
                dhb = dh_tiles[bi]
                dh_v = dhb[:, 2 * qb:2 * qb + 2, :]
                rht8 = s_['rht8']
                # the last chunk borrows the hg bank: psM's single buffer
                # would gate its m-group on the previous chunk's tanh
                pool = psH if use_psh else psM
                mpre = pool.tile([128, S2], f32,
                                 tag="hg" if use_psh else "mpre",
                                 name=f"mpre_{c}")
                nc.tensor.matmul(mpre[:], s_['xt_c'], WhTv,
                                 start=True, stop=False)
                # diagonal term 0.8*(r*h) exact in bf16
                nc.tensor.matmul(mpre[:], id08, s_['rh'][:],
                                 start=False, stop=False)
                for i in range(2):
                    sl = slice(2 * i, 2 * i + 2)
                    nc.tensor.matmul(mpre[:], rht8[:, sl, :], Vh8[:, sl, :],
                                     start=False, stop=(i == 1),
                                     perf_mode=DR)
                m = wpool.tile([128, S2], bf16, tag="m")
                t1 = wpool.tile([128, S2], bf16, tag="t1")
                dh_flat = dh_v.rearrange("p a b -> p (a b)")
                if split:
                    # halve the tail chain so ACT/DVE/DMA pipeline and the
                    # final store issues earlier (used on the last chunks,
                    # which close their store blocks)
                    assert qb == nb - 1
                    for hi, hs in enumerate((slice(0, S), slice(S, S2))):
                        nc.scalar.activation(m[:, hs], mpre[:, hs], Tanh)
                        nc.vector.tensor_sub(t1[:, hs], m[:, hs],
                                             s_['hg2'][:, hs])
                        nc.vector.tensor_mul(dh_flat[:, hs], s_['zc'][:, hs],
                                             t1[:, hs])
                        lo = 0 if hi == 0 else 2 * qb + 1
                        hh = 2 * qb + 1 if hi == 0 else 2 * nb
                        # alternate HWDGE rings so the two half-stores
                        # don't serialize on one engine's issue cost; the
                        # last chunk flips the order so its final store
                        # lands on whichever ring is free
                        flip = (c == issue_order[-1])
                        eng = nc.scalar if (hi == 0) == flip else nc.sync
                        eng.dma_start(
                            dh_out[:, 2 * cs + lo:2 * cs + hh, :],
                            dhb[:, lo:hh, :])
                    return
                nc.scalar.activation(m[:], mpre[:], Tanh)
                nc.vector.tensor_sub(t1[:], m[:], s_['hg2'])
                nc.vector.tensor_mul(dh_flat, s_['zc'], t1[:])
                # ship deltas to host in per-block batches (sync HWDGE)
                if qb == nb - 1:
                    nc.sync.dma_start(
                        dh_out[:, 2 * cs:2 * (cs + nb), :], dhb[:])

            # wave-2 chunks (k1, k1+1) depend only on chunks 0/1; interleave
            # them among late wave-1 chunks. 3-stage software pipeline keeps
            # the PE FIFO from head-of-line blocking on ACT/DVE stages.
            issue_order = [0, 1, 2, 3, k1, k1 + 1, 4]
            assert sorted(issue_order) == list(range(n_chunks))
            for i, c in enumerate(issue_order):
                stage_A(c)
                if i >= 1:
                    stage_B1(issue_order[i - 1])
                if i >= 2:
                    stage_B2(issue_order[i - 2])

            # dependency-free filler matmuls slotted into the pipeline-drain
            # waits: the PE idles here on ACT/DVE chains long enough for the
            # HAM MID window to re-throttle the clock, which would make the
            # final real matmuls run at 1.2 GHz
            fill_ps = psH.tile([128, S2], f32, tag="hg", name="filler")

            def filler(n):
                for _ in range(n):
                    nc.tensor.matmul(fill_ps[:], wsrc[:, 0:128], wsrc[:],
                                     start=True, stop=True)

            stage_B1(issue_order[-1])
            filler(3)
            stage_B2(issue_order[-2], split=True)
            filler(3)
            stage_B2(issue_order[-1], split=True, use_psh=True)

    nc.compile()
    return nc


def _in_map(hd, core):
    pc = hd['per_core'][core]
    return {
        "tI": pc['tI'], "tA": pc['tA'], "t8a": pc['t8a'],
        "t8b": pc['t8b'], "t8c": pc['t8c'],
        "tF1": pc['tF1'], "tF2": pc['tF2'],
    }


def _run(hd, nc, trace=False):
    from concourse.bass_utils import run_bass_kernel_spmd
    return run_bass_kernel_spmd(nc, [_in_map(hd, c) for c in range(8)],
                                list(range(8)), trace=trace)


def _assemble(hd, dh_cores, table0):
    """Apply device deltas (rows never cross cores), then finish the tail
    waves on host (same-level steps never share a row -> batched GEMMs)."""
    n_chunks = hd['n_chunks']
    out = table0.astype(np.float32).copy()
    for cidx in range(8):
        dh = np.ascontiguousarray(dh_cores[cidx].transpose(1, 0, 2))
        dh = dh.reshape(n_chunks, 2, CHUNK, S).transpose(0, 2, 1, 3)
        dh = dh.reshape(hd['T_pad'] * 2, S)
        pc = hd['per_core'][cidx]
        rows = np.stack([pc['ob'], pc['op']], axis=1).reshape(-1)
        valid = rows >= 0
        np.add.at(out, rows[valid], dh[valid])

    hs = np.asarray(hd['host_steps'], np.int64)
    if len(hs):
        x, b, p = hd['x'], hd['b'], hd['p']
        Wz, Wr, Wh = hd['Wz'], hd['Wr'], hd['Wh']
        Uz, Ur, Uh = hd['Uz'], hd['Ur'], hd['Uh']
        bz, br, bh = hd['bz'], hd['br'], hd['bh']
        levs = hd['lev'][hs]
        for L in np.unique(levs):
            ts = hs[levs == L]
            H = np.concatenate([out[b[ts]], out[p[ts]]], axis=1)
            Z = 1 / (1 + np.exp(-(x[ts] @ Wz.T + H @ Uz.T + bz)))
            R = 1 / (1 + np.exp(-(x[ts] @ Wr.T + H @ Ur.T - br)))
            M = np.tanh(x[ts] @ Wh.T + (R * H) @ Uh.T + bh)
            dh = (1.0 - Z) * (M - H)
            np.add.at(out, b[ts], dh[:, :S])
            np.add.at(out, p[ts], dh[:, S:])
    return out


def kernel(**inputs):
    x = np.asarray(inputs['x'], dtype=np.float32)
    b = np.asarray(inputs['b'])
    p = np.asarray(inputs['p'])
    table0 = np.asarray(inputs['table0'], dtype=np.float32)

    hd = _build_host_data(
        x, b, p,
        np.asarray(inputs['Wz'], np.float32), np.asarray(inputs['Wr'], np.float32),
        np.asarray(inputs['Wh'], np.float32), np.asarray(inputs['Uz'], np.float32),
        np.asarray(inputs['Ur'], np.float32), np.asarray(inputs['Uh'], np.float32),
        np.asarray(inputs['bz'], np.float32), np.asarray(inputs['br'], np.float32),
        np.asarray(inputs['bh'], np.float32), table0)

    nc = _build_nc(hd)
    res = _run(hd, nc)
    dh_cores = [np.asarray(res.results[c]["dh"], np.float32) for c in range(8)]
    return _assemble(hd, dh_cores, table0)


if __name__ == "__main__":
    d = np.load('/tmp/ref_inputs.npz')
    inputs = {k: d[k] for k in d.files}
    got = kernel(**inputs)
    exp = np.load('/tmp/ref_out_np.npy')
    err = np.abs(got - exp).max()
    print("abs err:", err, "rel:", err / np.abs(exp).max())
